# revision 33
# baseline (speedup 1.0000x reference)
"""Trainium2 Bass kernel for nn_Attn_45423574123081 (sparse_attention).

Computes, for inputs enc [B=32, L=1024, D=64], W [64, 64], b [64]:
    energy = enc @ W.T + b                       # [B, L, D]
    scores[t, b, j] = energy[b, j] . enc[b, t]   # [T=1024, B, L]
    scores[t, :, t] = 0
    out = softmax(scores, axis=-1)

Sharding: data-parallel over batch; 4 batches per core on 8 NeuronCores.

v2.1 design notes (from v2 trace analysis):
 * Output DMA'd as bf16 (half the bytes; host converts to f32).  The
   harness gate is 2e-2 relative error; bf16 probabilities add ~2e-3.
 * Single-f16 W and G: K=64 matmuls cost the same as K=128 and the f16
   quantization of E dominates the error anyway.
 * enc is cast f32->f16 in-flight by the SWDGE (gpsimd) input DMA.
 * GpSimd runs NO tensor ops (measured 14.8us per [128,1024] op, and it
   locks DVE out of SBUF, poisoning concurrent DVE work) -- only DMAs.
 * Paired PE transposes: E^T is built with 4 [128,128] transposes per
   batch instead of 8 [128,64] ones; odd t-chunks land on partitions
   64-127, so G^T and b-col are row-duplicated (tiny SWDGE copies) and
   each S matmul picks the matching 64-partition half.
 * Softmax: S = E16 @ G^T in PSUM f32, DVE writes -c_t (c = E.b) on the
   diagonal, ScalarE exp -> SBUF bf16 (bf16 has f32 range: no overflow
   without max-subtraction).  Row sums mostly via the ACT accumulator
   (287ns readout); a few chunks per batch use a DVE copy-with-accum
   pass instead to balance the two engines.  Normalize = one DVE
   multiply by the reciprocal row sum.
"""

import numpy as np

_B, _L, _D, _T = 32, 1024, 64, 1024
_N_CORES = 8
_BPC = _B // _N_CORES  # batches per core

# Per-batch chunk sum modes (8 chunks per batch):
#   'A' = ScalarE accumulator sums (measured +~370ns on the ACTIVATE plus
#         a ~350ns READ_ACCUMULATOR readout)
#   'T' = DVE tensor_tensor_reduce: fold the two 512 halves with add and
#         row-reduce in one ~0.6us op (cheaper than CACHE_REDUCE's 1x
#         full-width pass)
_MODES = ["A", "A", "A", "A", "A", "A", "A", "A"]

_compiled_nc = None


def _build():
    global _compiled_nc
    if _compiled_nc is not None:
        return _compiled_nc

    import concourse.bacc as bacc
    import concourse.mybir as mybir
    from concourse import tile, masks

    dt = mybir.dt
    AF = mybir.ActivationFunctionType
    ALU = mybir.AluOpType

    nc = bacc.Bacc(
        "TRN2",
        target_bir_lowering=False,
        debug=False,
        enable_asserts=False,
        num_devices=_N_CORES,
    )
    enc_d = nc.dram_tensor("enc", [_BPC, _L, _D], dt.float32, kind="ExternalInput")
    w_d = nc.dram_tensor("w", [_D, _D], dt.float32, kind="ExternalInput")
    b_d = nc.dram_tensor("bias", [_D], dt.float32, kind="ExternalInput")
    # host-provided identity masks: building them on GpSimd costs ~1us of
    # critical-path time at startup; a 48 KB DMA is cheaper
    id16_d = nc.dram_tensor("ident16", [128, 128], dt.float16, kind="ExternalInput")
    id8_d = nc.dram_tensor("ident8", [128, 128], dt.int8, kind="ExternalInput")
    out_d = nc.dram_tensor("out", [_T, _BPC, _L], dt.bfloat16, kind="ExternalOutput")

    with tile.TileContext(nc) as tc:
        with (
            tc.tile_pool(name="const", bufs=1) as cpool,
            tc.tile_pool(name="encp", bufs=2) as encpool,
            tc.tile_pool(name="etp", bufs=2) as etpool,
            tc.tile_pool(name="gtp", bufs=2) as gtpool,
            tc.tile_pool(name="ebp", bufs=2) as ebpool,
            tc.tile_pool(name="expp", bufs=6) as exppool,
            tc.tile_pool(name="outp", bufs=3) as outpool,
            tc.tile_pool(name="sump", bufs=2) as sumpool,
            tc.tile_pool(name="scrp", bufs=2) as scrpool,
            tc.tile_pool(name="ps_s", bufs=3, space="PSUM") as ps_s_pool,
            tc.tile_pool(name="ps_p", bufs=2, space="PSUM") as ps_p_pool,
        ):
            # Dummy exp at t=0: walrus inserts ACT_TABLE_LOAD right before
            # the first Exp activation, which otherwise sits behind the
            # first chunk's semaphore wait and lands on the critical path.
            warm = cpool.tile([1, 2], dt.float32)
            nc.vector.memset(warm[:], 0.0)
            nc.scalar.activation(warm[:, 0:1], warm[:, 1:2], AF.Exp)

            # Load enc for batch 0 FIRST: the SWDGE queue must not make the
            # critical-path input load wait behind the identity-mask memsets.
            enc16_b0 = encpool.tile([128, 8 * _D], dt.float16, tag="enc16")
            for h in range(2):
                nc.gpsimd.dma_start(
                    enc16_b0[:, h * 4 * _D : (h + 1) * 4 * _D].rearrange(
                        "p (n d) -> p n d", n=4
                    ),
                    enc_d[0].rearrange("(n p) d -> p n d", p=128)[
                        :, h * 4 : (h + 1) * 4
                    ],
                )

            # sync-queue order: w -> ident16 -> b -> ident8, so the W/b
            # prep chains (which feed G and eb) start as early as possible
            w_sb = cpool.tile([_D, _D], dt.float32)
            nc.sync.dma_start(w_sb[:], w_d[:])
            ident16 = cpool.tile([128, 128], dt.float16)
            nc.sync.dma_start(ident16[:], id16_d[:])
            b_row0 = cpool.tile([1, _D], dt.float32)
            nc.sync.dma_start(b_row0[:], b_d[:].unsqueeze(0))
            ident_i8 = cpool.tile([128, 128], dt.int8)
            nc.sync.dma_start(ident_i8[:], id8_d[:])

            # --- W^T f16 [64, 64] (lhsT for G^T = W @ E^T)
            w16 = cpool.tile([_D, _D], dt.float16)
            nc.vector.tensor_copy(w16[:], w_sb[:])
            ps_w = ps_p_pool.tile([_D, _D], dt.float16, tag="ps_p")
            nc.tensor.transpose(ps_w[:], w16[:], ident16[: _D, : _D])
            # W^T on both partition halves (stationary for either parity)
            w16t = cpool.tile([128, _D], dt.float16)
            nc.vector.tensor_copy(w16t[: _D, :], ps_w[:])
            nc.vector.tensor_copy(w16t[_D :, :], ps_w[:])

            # --- b as f16 column, duplicated to partitions 64-127
            b16r = cpool.tile([1, _D], dt.float16)
            nc.vector.tensor_copy(b16r[:], b_row0[:])
            ps_b = ps_p_pool.tile([_D, 1], dt.float16, tag="ps_p")
            nc.tensor.transpose(ps_b[:], b16r[:], ident16[:1, :1])
            b2c = cpool.tile([128, 1], dt.float16)
            nc.vector.tensor_copy(b2c[: _D, :], ps_b[:])
            nc.sync.dma_start(b2c[_D :, :], b2c[: _D, :])

            def prep_load(bb):
                """enc f32 DRAM -> f16 SBUF, cast in-flight on SWDGE."""
                enc16 = encpool.tile([128, 8 * _D], dt.float16, tag="enc16")
                nc.gpsimd.dma_start(
                    enc16[:].rearrange("p (n d) -> p n d", n=8),
                    enc_d[bb].rearrange("(n p) d -> p n d", p=128),
                )
                return enc16

            def prep_tr(bb, enc16):
                """E16^T as [128, 512]: pair q holds t-chunk 2q on
                partitions 0-63 and t-chunk 2q+1 on partitions 64-127."""
                ps_et = ps_p_pool.tile([128, 512], dt.float16, tag="ps_p")
                for q in range(4):
                    nc.tensor.transpose(
                        ps_et[:, q * 128 : (q + 1) * 128],
                        enc16[:, (2 * q) * _D : (2 * q + 2) * _D],
                        ident16[:],
                    )
                et16 = etpool.tile([128, 512], dt.float16, tag="et16")
                nc.vector.tensor_copy(et16[:], ps_et[:])
                return et16

            def et_sl(et16, c):
                """lhsT slice [64, 128] for t-chunk c."""
                p0 = _D * (c % 2)
                q = c // 2
                return et16[p0 : p0 + _D, q * 128 : (q + 1) * 128]

            def prep_g(bb, et16):
                """G^T = W @ E16^T, f16, by t-chunk parity (et16 keeps even
                chunks on partitions 0-63, odd on 64-127); the PSUM->SBUF
                copy scatters each parity's four 128-col groups into their
                j positions.  Then row-duplicate to partitions 64-127."""
                gt2 = gtpool.tile([128, _L], dt.float16, tag="gt2")
                for par in range(2):
                    p0 = _D * par
                    ps_gt = ps_p_pool.tile([_D, 512], dt.float32, tag="ps_p")
                    nc.tensor.matmul(
                        ps_gt[:],
                        w16t[p0 : p0 + _D, :],
                        et16[p0 : p0 + _D, :],
                        start=True,
                        stop=True,
                    )
                    dst = gt2[: _D, :].rearrange("p (n j) -> p n j", n=8)[
                        :, par :: 2
                    ]
                    nc.vector.tensor_copy(
                        dst, ps_gt[:].rearrange("p (n j) -> p n j", n=4)
                    )
                return gt2

            def dup_g(gt2):
                """Row-duplicate G^T to partitions 64-127 (needed by odd
                t-chunks only, so this runs off chunk 0's critical path)."""
                nc.gpsimd.dma_start(gt2[_D :, :], gt2[: _D, :])

            def prep_eb(bb, et16):
                """-c = -(E16 . b) as [128, 8] f32 (chunk-major columns)."""
                ps_eb = ps_p_pool.tile([128, 8], dt.float32, tag="ps_p")
                for c in range(8):
                    p0 = _D * (c % 2)
                    nc.tensor.matmul(
                        ps_eb[:, c : c + 1],
                        et_sl(et16, c),
                        b2c[p0 : p0 + _D, :],
                        start=True,
                        stop=True,
                    )
                ebn = ebpool.tile([128, 8], dt.float32, tag="ebn")
                nc.vector.tensor_scalar_mul(ebn[:], ps_eb[:], -1.0)
                return ebn

            def chunk(bb, i, et16, gt2, ebn, sums):
                """One t-block: matmul, diag write, exp, row sum.
                Returns the bf16 exp tile (normalized later, per pair)."""
                mode = _MODES[i]
                bsl = slice(i * 128, (i + 1) * 128)
                p0 = _D * (i % 2)
                ps = ps_s_pool.tile([128, _L], dt.float32, tag="ps_s")
                c_diag = i // 4
                for c in (c_diag, 1 - c_diag):
                    sl = slice(c * 512, (c + 1) * 512)
                    nc.tensor.matmul(
                        ps[:, sl],
                        et_sl(et16, i),
                        gt2[p0 : p0 + _D, sl],
                        start=True,
                        stop=True,
                    )
                    if c == c_diag:
                        nc.vector.copy_predicated(
                            ps[:, bsl],
                            ident_i8[:],
                            ebn[:, i : i + 1].to_broadcast([128, 128]),
                        )
                exp_sb = exppool.tile([128, _L], dt.bfloat16, tag="exp")
                scol = sums[:, i : i + 1]
                if mode == "A":
                    nc.scalar.activation(exp_sb[:], ps[:], AF.Exp, accum_out=scol)
                else:
                    nc.scalar.activation(exp_sb[:], ps[:], AF.Exp)
                    scr = scrpool.tile([128, 512], dt.bfloat16, tag="scr")
                    nc.vector.tensor_tensor_reduce(
                        scr[:],
                        exp_sb[:, 0:512],
                        exp_sb[:, 512:1024],
                        1.0,
                        0.0,
                        ALU.add,
                        ALU.add,
                        accum_out=scol,
                    )
                return exp_sb

            def finish_pair(bb, q, exps, sums, recips):
                """Reciprocal for chunks 2q/2q+1, normalize, DMA out."""
                pr = slice(2 * q, 2 * q + 2)
                nc.vector.reciprocal(recips[:, pr], sums[:, pr])
                out16 = outpool.tile([128, 2 * _L], dt.bfloat16, tag="o16")
                for h in range(2):
                    i = 2 * q + h
                    nc.vector.tensor_scalar_mul(
                        out16[:, h * _L : (h + 1) * _L],
                        exps[i][:],
                        recips[:, i : i + 1],
                    )
                dst = (
                    out_d[256 * q : 256 * (q + 1), bb : bb + 1, :]
                    .squeeze(1)
                    .rearrange("(h p) j -> p h j", p=128)
                )
                nc.sync.dma_start(dst, out16[:].rearrange("p (h j) -> p h j", h=2))

            def finish_chunk(bb, i, exp_sb, sums, recips):
                """Tail-latency variant for the last batch: normalize and
                ship each 256 KB t-block as soon as its sum lands."""
                nc.vector.reciprocal(recips[:, i : i + 1], sums[:, i : i + 1])
                out16 = outpool.tile([128, _L], dt.bfloat16, tag="o16s")
                nc.vector.tensor_scalar_mul(out16[:], exp_sb[:], recips[:, i : i + 1])
                dst = (
                    out_d[128 * i : 128 * (i + 1), bb : bb + 1, :]
                    .squeeze(1)
                    .rearrange("(one p) j -> p one j", p=128)
                    .squeeze(1)
                )
                eng = nc.scalar if i == 7 else nc.sync
                eng.dma_start(dst, out16[:])

            # --- software-pipelined emission ---------------------------------
            enc = [None] * _BPC
            et = [None] * _BPC
            gt = [None] * _BPC
            eb = [None] * _BPC
            enc[0] = enc16_b0
            et[0] = prep_tr(0, enc[0])
            gt[0] = prep_g(0, et[0])
            eb[0] = prep_eb(0, et[0])
            dup_g(gt[0])
            enc[1] = prep_load(1)

            pending = []
            for bb in range(_BPC):
                sums = sumpool.tile([128, 8], dt.float32, tag="sums")
                recips = sumpool.tile([128, 8], dt.float32, tag="recips")
                exps = [None] * 8
                last = bb == _BPC - 1
                if bb == 0:
                    # batch 0: even t-chunks first -- odd chunks need the
                    # G^T row-duplication (gated on both G casts + a SWDGE
                    # copy, ready ~6us after the first even matmul could
                    # run); ship each chunk individually
                    for i in (0, 2, 4, 6, 1, 3, 5, 7):
                        exps[i] = chunk(bb, i, et[bb], gt[bb], eb[bb], sums)
                        finish_chunk(bb, i, exps[i], sums, recips)
                        if i == 4:
                            with tc.high_priority(offset=-30):
                                et[1] = prep_tr(1, enc[1])
                        elif i == 1:
                            with tc.high_priority(offset=-30):
                                gt[1] = prep_g(1, et[1])
                        elif i == 3:
                            with tc.high_priority(offset=-30):
                                eb[1] = prep_eb(1, et[1])
                                dup_g(gt[1])
                        elif i == 5:
                            with tc.high_priority(offset=-30):
                                enc[2] = prep_load(2)
                    continue
                for i in range(8):
                    exps[i] = chunk(bb, i, et[bb], gt[bb], eb[bb], sums)
                    if pending:
                        # deferred pair finish: always emit it AFTER the next
                        # chunk's diagonal write so the in-order DVE queue
                        # never makes ScalarE wait on normalize work
                        pending.pop()()
                    if last and i >= 6:
                        # last pair: ship each 256 KB block immediately to
                        # shorten the end-of-kernel DMA drain
                        finish_chunk(bb, i, exps[i], sums, recips)
                    elif i % 2 == 1:
                        pending.append(
                            lambda bb=bb, q=i // 2, e=exps, s=sums, r=recips:
                                finish_pair(bb, q, e, s, r)
                        )
                    # negative-offset priority: the Tile scheduler list-
                    # schedules by priority among READY instructions; preps
                    # are ready long before they are needed and otherwise
                    # jump ahead of urgent diag-writes/normalizes
                    if bb + 1 < _BPC:
                        with tc.high_priority(offset=-30):
                            if i == 3:
                                et[bb + 1] = prep_tr(bb + 1, enc[bb + 1])
                            elif i == 5:
                                gt[bb + 1] = prep_g(bb + 1, et[bb + 1])
                            elif i == 6:
                                eb[bb + 1] = prep_eb(bb + 1, et[bb + 1])
                                dup_g(gt[bb + 1])
                    if bb + 2 < _BPC and i == 6:
                        with tc.high_priority(offset=-30):
                            enc[bb + 2] = prep_load(bb + 2)

    nc.compile()
    _compiled_nc = nc
    return nc


def _numpy_fallback(enc, W, b, tl):
    energy = np.einsum("bld,ed->ble", enc, W) + b
    scores = np.einsum("bjd,btd->tbj", energy, enc[:, :tl, :])
    t_idx = np.arange(tl)
    scores[t_idx, :, t_idx] = 0.0
    m = scores.max(axis=-1, keepdims=True)
    e = np.exp(scores - m)
    return (e / e.sum(axis=-1, keepdims=True)).astype(np.float32)


def _run(encoder_outputs, W, b, target_length=1024, **run_kwargs):
    enc = np.ascontiguousarray(np.asarray(encoder_outputs, dtype=np.float32))
    Wn = np.ascontiguousarray(np.asarray(W, dtype=np.float32))
    bn = np.ascontiguousarray(np.asarray(b, dtype=np.float32))
    tl = int(target_length)
    if enc.shape != (_B, _L, _D) or tl != _T:
        return _numpy_fallback(enc, Wn, bn, tl), None

    from concourse.bass_utils import run_bass_kernel_spmd

    nc = _build()
    id16 = np.eye(128, dtype=np.float16)
    id8 = np.eye(128, dtype=np.int8)
    in_maps = [
        {
            "enc": enc[i * _BPC : (i + 1) * _BPC],
            "w": Wn,
            "bias": bn,
            "ident16": id16,
            "ident8": id8,
        }
        for i in range(_N_CORES)
    ]
    res = run_bass_kernel_spmd(nc, in_maps, list(range(_N_CORES)), **run_kwargs)
    out = np.concatenate(
        [np.asarray(res.results[i]["out"]) for i in range(_N_CORES)], axis=1
    ).astype(np.float32)
    return out, res


def kernel(encoder_outputs, W, b, target_length=1024):
    out, _ = _run(encoder_outputs, W, b, target_length)
    return out


def kernel_profiled(encoder_outputs, W, b, target_length=1024):
    """Run with NTFF tracing; returns (output, BassKernelResults)."""
    return _run(encoder_outputs, W, b, target_length, trace=True)
